# revision 17
# baseline (speedup 1.0000x reference)
"""Expert-parallel MoE MLP kernel for Trainium2 (8 NeuronCores).

Problem: x[B=2,S=1024,H=1024] f32, expert_indices[B,S] int, 16 experts,
gate/up_proj[E,H,I], down_proj[E,I,H] (H=I=1024):
    out[n] = silu(x_n @ Wg[e_n]) * (x_n @ Wu[e_n]) @ Wd[e_n].T

Sharding: expert parallelism - core c owns experts {2c, 2c+1}. The host
groups tokens by expert (the "all-to-all dispatch" runs on host since the
kernel contract is full-input -> full-output), pads each expert's token
block to a 16-multiple capacity, and each core runs dense per-expert GEMMs.

All operands are bf16 (rel err ~4e-3 vs the 2e-2 gate): 12.6 MB of
mandatory weight traffic per core.

The device program is RAW bass (no Tile framework). Profile-driven
design (v5):
  - the kernel is bound by SDMA per-engine line rate (~23 GB/s/engine at
    2 KB descriptors, 25.3 at 4 KB, 25.9 at 8 KB; descriptor = the
    per-partition contiguous DRAM run). The weight bulk streams as 1 MB
    chunks with 8 KB runs; fine grain only where the pipeline needs it:
    expert 0's first gate chunks are 0.5 MB so the PE starts early, and
    the last expert's down stream ends in single-j-tile chunks so the
    post-stream dependency is just 8 matmuls + 1 PSUM copy + 1 36 KB
    store
  - every weight chunk has its OWN completion semaphore and ALL configs
    are pre-issued on the SP ring with no pacing waits (each chunk has a
    dedicated SBUF slot -> no WAR hazard; per-ring FIFO keeps order) -
    the stream runs gapless at line rate
  - x (both experts, one 4.6 KB-run DMA) is SP-ring entry 0: the PE's
    first dependency, done ~4 us in
  - PE chases the stream h-outer; one 8-bank PSUM pool: gate accumulates
    into 8 banks, up reuses them after per-bank silu consumption, down
    gets bank j per output j-tile. The first down j-tile accumulates
    k-tiles in DVE-mul completion order (per-k waits) so the PE overlaps
    the serial 8-mul chain instead of idling ~2 us behind it
  - output stores ride the Act queue as one 1728 B-run store for j0-5
    plus a small tail; the final j-tile store goes on the (by then idle)
    SP queue
"""

import math

import numpy as np

E = 16
H = 1024
HT = 8           # H / 128 partition tiles
N_CORES = 8
EPC = E // N_CORES   # experts per core
NSD = 8          # down_proj j-tiles per expert (chunked into j-blocks)

# chunk h/j-blocks, chosen for >=8 KB per-partition DRAM runs (higher
# SDMA per-engine line rate) except where the pipeline needs fine grain:
# the first gate chunks of expert 0 are small so the PE starts early,
# and the LAST expert's down stream ends with single-tile blocks so the
# kernel tail after the last weight byte is minimal
_GBLK0 = [(0, 2), (2, 4), (4, 8)]        # gate, expert 0 (PE warm-up)
_GBLK = [(0, 8)]                         # gate, experts 1+ / up, all: 2 MB
_DBLK = [(0, 8)]                         # down, experts 0..EPC-2: 2 MB
_DBLK_LAST = [(0, 4), (4, 6), (6, 7), (7, 8)]   # down, expert EPC-1

_NC_CACHE = {}


def _build_nc_raw(pio: int, act: str = "Silu"):
    """Raw-bass SPMD program. pio: padded token count, multiple of 16,
    <= 160."""
    from concourse import bacc, mybir
    from concourse.bass import ts

    f32 = mybir.dt.float32
    bf16 = mybir.dt.bfloat16
    SILU = getattr(mybir.ActivationFunctionType, act)
    assert 3 * pio * 4 <= 2048

    nc = bacc.Bacc("TRN2", target_bir_lowering=False, debug=False,
                   num_devices=N_CORES)
    w = nc.dram_tensor("w", [EPC, 2, 128, HT, H], bf16, kind="ExternalInput")
    wd = nc.dram_tensor("wd", [EPC, 128, NSD, HT, 128], bf16,
                        kind="ExternalInput")
    xt = nc.dram_tensor("xt", [128, EPC, HT, pio], bf16, kind="ExternalInput")
    out = nc.dram_tensor("out", [EPC, 128, HT, pio], bf16,
                         kind="ExternalOutput")

    def gblk(e):
        return _GBLK0 if e == 0 else _GBLK

    def dblk(e):
        return _DBLK_LAST if e == EPC - 1 else _DBLK

    # flat chunk list in stream order; values: (expert, kind, lo, hi)
    chunks = []
    for e in range(EPC):
        for (lo, hi) in gblk(e):
            chunks.append((e, 0, lo, hi))                  # gate h-tiles
        for (lo, hi) in _GBLK:
            chunks.append((e, 1, lo, hi))                  # up h-tiles
        for (j0, j1) in dblk(e):
            chunks.append((e, 2, j0, j1))                  # down j-tiles
    NW = len(chunks)
    cidx = {c: k for k, c in enumerate(chunks)}
    # h -> owning chunk (lo, hi), per expert and projection
    hchunk = {}
    for e in range(EPC):
        for (lo, hi) in gblk(e):
            for h in range(lo, hi):
                hchunk[(e, 0, h)] = (lo, hi)
        for (lo, hi) in _GBLK:
            for h in range(lo, hi):
                hchunk[(e, 1, h)] = (lo, hi)

    import contextlib
    with contextlib.ExitStack() as st:
        s_ws = [st.enter_context(nc.semaphore(f"s_w{i}")) for i in range(NW)]
        s_x = [st.enter_context(nc.semaphore("s_x"))]
        s_g = st.enter_context(nc.semaphore("s_g"))   # gate region done (PE)
        s_u = st.enter_context(nc.semaphore("s_u"))   # up region done (PE)
        s_s = st.enter_context(nc.semaphore("s_s"))   # silu done (Act)
        s_m = st.enter_context(nc.semaphore("s_m"))   # inter mul done (DVE)
        s_d = st.enter_context(nc.semaphore("s_d"))   # down region done (PE)
        s_c = st.enter_context(nc.semaphore("s_c"))   # out copy done (DVE)
        s_o = st.enter_context(nc.semaphore("s_o"))   # out stores (+16 each)
        s_o2 = st.enter_context(nc.semaphore("s_o2"))  # final out on SP queue
        wgu = st.enter_context(
            nc.sbuf_tensor("wgu", [128, 2 * EPC, HT, H], bf16))
        wdn = st.enter_context(
            nc.sbuf_tensor("wdn", [128, NSD * EPC, HT, 128], bf16))
        x_sb = st.enter_context(
            nc.sbuf_tensor("x_sb", [128, EPC, HT, pio], bf16))
        g_sb = st.enter_context(
            nc.sbuf_tensor("g_sb", [128, EPC, HT, pio], f32))
        i_sb = st.enter_context(
            nc.sbuf_tensor("i_sb", [128, EPC, HT, pio], bf16))
        o_sb = st.enter_context(
            nc.sbuf_tensor("o_sb", [128, EPC, HT, pio], bf16))
        # one 8-bank PSUM pool; only one accumulation group may be open
        # per bank, so gate/up/down reuse banks with explicit WAR waits
        p8 = st.enter_context(nc.psum_tensor("p8", [128, 8, 512], f32))

        def reg(i):
            return p8[:, i, 0:pio]

        # kernel issues no GpSimd work: skip its expensive DGE drain
        # and use the cheaper sem-only end barrier
        with nc.Block(no_gpsimd_drain=True) as block:

            def w_cfg(eng, k):
                e, kind, lo, hi = chunks[k]
                if kind < 2:
                    eng.dma_start(wgu[:, e * 2 + kind, lo:hi],
                                  w[e, kind, :, lo:hi, :]
                                  ).then_inc(s_ws[k], 16)
                else:
                    eng.dma_start(
                        wdn[:, e * NSD + lo:e * NSD + hi],
                        wd[e, :, lo:hi]).then_inc(s_ws[k], 16)

            @block.sync
            def _(sync):
                # x (both experts, one 4.6 KB-run DMA) first - the PE's
                # first dependency - then the weight stream: every config
                # pre-issued, no pacing
                sync.dma_start(x_sb[:, :], xt[:, :]).then_inc(s_x[0], 16)
                for k in range(NW):
                    w_cfg(sync, k)
                # final j-tile ships from the (by now idle) SP queue so it
                # doesn't wait behind the Act queue's previous store
                sync.wait_ge(s_c, 8 * EPC)
                sync.dma_start(out[EPC - 1, :, HT - 1:HT, :],
                               o_sb[:, EPC - 1, HT - 1:HT]).then_inc(s_o2, 16)

            @block.tensor
            def _(tensor):
                tensor.wait_ge(s_x[0], 16)
                for e in range(EPC):
                    # gate, h-outer, chasing chunk arrival
                    for h in range(HT):
                        blk = hchunk[(e, 0, h)]
                        if h == blk[0]:
                            tensor.wait_ge(
                                s_ws[cidx[(e, 0) + blk]], 16)
                        for i in range(HT):
                            if h == 0 and e > 0:
                                # bank i WAR: expert e-1's down j-tile i
                                # must be copied out first
                                tensor.wait_ge(s_c, 8 * (e - 1) + i + 1)
                            mm = tensor.matmul(
                                reg(i), wgu[:, e * 2, h, ts(i, 128)],
                                x_sb[:, e, h],
                                start=(h == 0), stop=(h == HT - 1))
                            if h == HT - 1:
                                mm.then_inc(s_g)
                    # up (reuses gate's banks; region i waits silu[i])
                    for h in range(HT):
                        blk = hchunk[(e, 1, h)]
                        if h == blk[0]:
                            tensor.wait_ge(
                                s_ws[cidx[(e, 1) + blk]], 16)
                        for i in range(HT):
                            if h == 0:
                                tensor.wait_ge(s_s, 8 * e + i + 1)
                            mm = tensor.matmul(
                                reg(i), wgu[:, e * 2 + 1, h, ts(i, 128)],
                                x_sb[:, e, h],
                                start=(h == 0), stop=(h == HT - 1))
                            if h == HT - 1:
                                mm.then_inc(s_u)
                    # down, j-block chunks (bank j <- output j-tile). The
                    # first j-tile chases the DVE mul chain per-k (mul[k]
                    # wrote i_sb[k] and freed bank k) instead of waiting
                    # for all 8 muls; the second block carries the full
                    # wait, later blocks are implicitly ordered behind it.
                    for bi, (j0, j1) in enumerate(dblk(e)):
                        tensor.wait_ge(s_ws[cidx[(e, 2, j0, j1)]], 16)
                        if bi == 1:
                            tensor.wait_ge(s_m, 8 * (e + 1))
                        for j in range(j0, j1):
                            for k in range(HT):
                                if bi == 0 and j == j0:
                                    tensor.wait_ge(s_m, 8 * e + k + 1)
                                mm = tensor.matmul(
                                    p8[:, j, 0:pio],
                                    wdn[:, e * NSD + j, k, :],
                                    i_sb[:, e, k],
                                    start=(k == 0), stop=(k == HT - 1))
                                if k == HT - 1:
                                    mm.then_inc(s_d)

            @block.scalar
            def _(scalar):
                n_st = 0
                for e in range(EPC):
                    for i in range(HT):
                        scalar.wait_ge(s_g, 8 * e + i + 1)
                        scalar.activation(g_sb[:, e, i], reg(i),
                                          SILU).then_inc(s_s)
                    # stores: one 1728 B-run store for j0-5, then the
                    # 6..8 tail (the last expert's final j-tile ships
                    # from the SP queue instead)
                    jsplit = ([(0, 6), (6, 8)] if e < EPC - 1
                              else [(0, 6), (6, 7)])
                    for (j0, j1) in jsplit:
                        scalar.wait_ge(s_c, 8 * e + j1)
                        scalar.dma_start(
                            out[e, :, j0:j1, :],
                            o_sb[:, e, j0:j1]).then_inc(s_o, 16)
                        n_st += 1
                scalar.wait_ge(s_o, 16 * n_st)       # drain output stores
                scalar.wait_ge(s_o2, 16)

            @block.vector
            def _(vector):
                for e in range(EPC):
                    for i in range(HT):
                        vector.wait_ge(s_s, 8 * e + i + 1)
                        vector.wait_ge(s_u, 8 * e + i + 1)
                        vector.tensor_mul(i_sb[:, e, i], g_sb[:, e, i],
                                          reg(i)).then_inc(s_m)
                    for j in range(HT):
                        vector.wait_ge(s_d, 8 * e + j + 1)
                        vector.tensor_copy(o_sb[:, e, j],
                                           p8[:, j, 0:pio]).then_inc(s_c)

    nc.compile()
    return nc


def _get_nc(pio: int):
    if pio not in _NC_CACHE:
        _NC_CACHE[pio] = _build_nc_raw(pio)
    return _NC_CACHE[pio]


_ROUND_CAP = 160          # max tokens/expert per round (3 PSUM regions/bank)


def _kernel_once(x, expert_indices, gate_proj, up_proj, down_proj):
    import ml_dtypes
    from concourse.bass_utils import run_bass_kernel_spmd

    bf16 = np.dtype(ml_dtypes.bfloat16)
    x = np.ascontiguousarray(x, dtype=np.float32)
    b, s, h = x.shape
    assert (h, gate_proj.shape) == (H, (E, H, H)), (x.shape, gate_proj.shape)

    n = b * s
    xf = x.reshape(n, h)
    idx = np.asarray(expert_indices).reshape(n).astype(np.int64)

    order = np.argsort(idx, kind="stable")       # token ids grouped by expert
    counts = np.bincount(idx, minlength=E)
    starts = np.zeros(E + 1, dtype=np.int64)
    np.cumsum(counts, out=starts[1:])
    maxc = int(counts.max())
    assert maxc <= _ROUND_CAP
    pio = max(16, 16 * math.ceil(maxc / 16))

    # per-core weight packing (bf16, partition-major)
    wr = np.stack([gate_proj, up_proj], axis=1).astype(bf16) \
        .reshape(N_CORES, EPC, 2, HT, 128, H).transpose(0, 1, 2, 4, 3, 5)
    wdr = np.ascontiguousarray(down_proj.transpose(0, 2, 1)).astype(bf16) \
        .reshape(N_CORES, EPC, HT, 128, NSD, 128).transpose(0, 1, 3, 4, 2, 5)
    in_maps = []
    tok_ids = []
    for c in range(N_CORES):
        xt_c = np.zeros((EPC, H, pio), dtype=np.float32)
        toks = []
        for le in range(EPC):
            e = c * EPC + le
            te = order[starts[e]:starts[e + 1]]
            toks.append(te)
            xt_c[le, :, :len(te)] = xf[te].T
        tok_ids.append(toks)
        in_maps.append({
            "w": np.ascontiguousarray(wr[c]),
            "wd": np.ascontiguousarray(wdr[c]),
            # device xt is [128, EPC, HT, pio] (partition-major)
            "xt": xt_c.astype(bf16).reshape(EPC, HT, 128, pio)
                  .transpose(2, 0, 1, 3).copy(),
        })

    nc = _get_nc(pio)
    res = run_bass_kernel_spmd(nc, in_maps, core_ids=list(range(N_CORES)))

    out = np.empty((n, h), dtype=np.float32)
    for c in range(N_CORES):
        o = res.results[c]["out"]                # [EPC, 128, HT, pio] bf16
        for le in range(EPC):
            te = tok_ids[c][le]
            oe = np.asarray(o[le]).astype(np.float32) \
                .transpose(1, 0, 2).reshape(h, pio)      # [H, pio]
            out[te] = oe[:, :len(te)].T
    return out.reshape(b, s, h)


def kernel(x, expert_indices, gate_proj, up_proj, down_proj):
    """Full-input -> full-output entry point.

    Tokens-per-expert above _ROUND_CAP (pathological skew; PSUM bound)
    are handled by running the device kernel in multiple rounds over
    disjoint token slices - outputs are per-token independent."""
    idx = np.asarray(expert_indices)
    counts = np.bincount(idx.reshape(-1).astype(np.int64), minlength=E)
    if counts.max() <= _ROUND_CAP:
        return _kernel_once(x, expert_indices, gate_proj, up_proj, down_proj)

    b, s, h = x.shape
    n = b * s
    xf = np.ascontiguousarray(x, dtype=np.float32).reshape(n, h)
    idxf = idx.reshape(n).astype(np.int64)
    order = np.argsort(idxf, kind="stable")
    starts = np.zeros(E + 1, dtype=np.int64)
    np.cumsum(np.bincount(idxf, minlength=E), out=starts[1:])
    out = np.empty((n, h), dtype=np.float32)
    rounds = math.ceil(counts.max() / _ROUND_CAP)
    for r in range(rounds):
        sel = np.concatenate([
            order[starts[e] + r * _ROUND_CAP:
                  min(starts[e] + (r + 1) * _ROUND_CAP, starts[e + 1])]
            for e in range(E)])
        if not len(sel):
            continue
        xr = xf[sel].reshape(1, len(sel), h)
        ir = idxf[sel].reshape(1, len(sel))
        out[sel] = _kernel_once(
            xr, ir, gate_proj, up_proj, down_proj).reshape(len(sel), h)
    return out.reshape(b, s, h)


# revision 18
# speedup vs baseline: 1.1252x; 1.1252x over previous
"""Expert-parallel MoE MLP kernel for Trainium2 (8 NeuronCores).

Problem: x[B=2,S=1024,H=1024] f32, expert_indices[B,S] int, 16 experts,
gate/up_proj[E,H,I], down_proj[E,I,H] (H=I=1024):
    out[n] = silu(x_n @ Wg[e_n]) * (x_n @ Wu[e_n]) @ Wd[e_n].T

Sharding: expert parallelism - core c owns experts {2c, 2c+1}. The host
groups tokens by expert (the "all-to-all dispatch" runs on host since the
kernel contract is full-input -> full-output), pads each expert's token
block to a 16-multiple capacity, and each core runs dense per-expert GEMMs.

All operands are bf16 (rel err ~4e-3 vs the 2e-2 gate): 12.6 MB of
mandatory weight traffic per core.

The device program is RAW bass (no Tile framework). Profile-driven
design (v5):
  - the kernel is bound by SDMA per-engine line rate (~23 GB/s/engine at
    2 KB descriptors, 25.3 at 4 KB, 25.9 at 8 KB; descriptor = the
    per-partition contiguous DRAM run). The weight bulk streams as 1 MB
    chunks with 8 KB runs; fine grain only where the pipeline needs it:
    expert 0's first gate chunks are 0.5 MB so the PE starts early, and
    the last expert's down stream ends in single-j-tile chunks so the
    post-stream dependency is just 8 matmuls + 1 PSUM copy + 1 36 KB
    store
  - every weight chunk has its OWN completion semaphore and ALL configs
    are pre-issued on the SP ring with no pacing waits (each chunk has a
    dedicated SBUF slot -> no WAR hazard; per-ring FIFO keeps order) -
    the stream runs gapless at line rate
  - x (both experts, one 4.6 KB-run DMA) is SP-ring entry 0: the PE's
    first dependency, done ~4 us in
  - PE chases the stream h-outer; one 8-bank PSUM pool: gate accumulates
    into 8 banks, up reuses them after per-bank silu consumption, down
    gets bank j per output j-tile. The first down j-tile accumulates
    k-tiles in DVE-mul completion order (per-k waits) so the PE overlaps
    the serial 8-mul chain instead of idling ~2 us behind it
  - output stores ride the Act queue as one 1728 B-run store for j0-5
    plus a small tail; the final j-tile store goes on the (by then idle)
    SP queue
"""

import math

import numpy as np

E = 16
H = 1024
HT = 8           # H / 128 partition tiles
N_CORES = 8
EPC = E // N_CORES   # experts per core
NSD = 8          # down_proj j-tiles per expert (chunked into j-blocks)

# chunk h/j-blocks, chosen for >=8 KB per-partition DRAM runs (higher
# SDMA per-engine line rate) except where the pipeline needs fine grain:
# the first gate chunks of expert 0 are small so the PE starts early,
# and the LAST expert's down stream ends with single-tile blocks so the
# kernel tail after the last weight byte is minimal
_GBLK0 = [(0, 2), (2, 4), (4, 6), (6, 8)]        # gate, expert 0
_GBLK = [(0, 2), (2, 4), (4, 6), (6, 8)]         # gate, experts 1+ / up
_DBLK = [(0, 2), (2, 4), (4, 6), (6, 8)]         # down, experts 0..EPC-2
_DBLK_LAST = [(0, 2), (2, 4), (4, 6), (6, 7), (7, 8)]   # down, expert EPC-1

_NC_CACHE = {}


def _build_nc_raw(pio: int, act: str = "Silu"):
    """Raw-bass SPMD program. pio: padded token count, multiple of 16,
    <= 160."""
    from concourse import bacc, mybir
    from concourse.bass import ts

    f32 = mybir.dt.float32
    bf16 = mybir.dt.bfloat16
    SILU = getattr(mybir.ActivationFunctionType, act)
    assert 3 * pio * 4 <= 2048

    nc = bacc.Bacc("TRN2", target_bir_lowering=False, debug=False,
                   num_devices=N_CORES)
    w = nc.dram_tensor("w", [EPC, 2, 128, HT, H], bf16, kind="ExternalInput")
    wd = nc.dram_tensor("wd", [EPC, 128, NSD, HT, 128], bf16,
                        kind="ExternalInput")
    xt = nc.dram_tensor("xt", [128, EPC, HT, pio], bf16, kind="ExternalInput")
    out = nc.dram_tensor("out", [EPC, 128, HT, pio], bf16,
                         kind="ExternalOutput")

    def gblk(e):
        return _GBLK0 if e == 0 else _GBLK

    def dblk(e):
        return _DBLK_LAST if e == EPC - 1 else _DBLK

    # flat chunk list in stream order; values: (expert, kind, lo, hi)
    chunks = []
    for e in range(EPC):
        for (lo, hi) in gblk(e):
            chunks.append((e, 0, lo, hi))                  # gate h-tiles
        for (lo, hi) in _GBLK:
            chunks.append((e, 1, lo, hi))                  # up h-tiles
        for (j0, j1) in dblk(e):
            chunks.append((e, 2, j0, j1))                  # down j-tiles
    NW = len(chunks)
    cidx = {c: k for k, c in enumerate(chunks)}
    # h -> owning chunk (lo, hi), per expert and projection
    hchunk = {}
    for e in range(EPC):
        for (lo, hi) in gblk(e):
            for h in range(lo, hi):
                hchunk[(e, 0, h)] = (lo, hi)
        for (lo, hi) in _GBLK:
            for h in range(lo, hi):
                hchunk[(e, 1, h)] = (lo, hi)

    import contextlib
    with contextlib.ExitStack() as st:
        s_ws = [st.enter_context(nc.semaphore(f"s_w{i}")) for i in range(NW)]
        s_x = [st.enter_context(nc.semaphore("s_x"))]
        s_g = st.enter_context(nc.semaphore("s_g"))   # gate region done (PE)
        s_u = st.enter_context(nc.semaphore("s_u"))   # up region done (PE)
        s_s = st.enter_context(nc.semaphore("s_s"))   # silu done (Act)
        s_m = st.enter_context(nc.semaphore("s_m"))   # inter mul done (DVE)
        s_d = st.enter_context(nc.semaphore("s_d"))   # down region done (PE)
        s_c = st.enter_context(nc.semaphore("s_c"))   # out copy done (DVE)
        s_o = st.enter_context(nc.semaphore("s_o"))   # out stores (+16 each)
        s_o2 = st.enter_context(nc.semaphore("s_o2"))  # final out on SP queue
        wgu = st.enter_context(
            nc.sbuf_tensor("wgu", [128, 2 * EPC, HT, H], bf16))
        wdn = st.enter_context(
            nc.sbuf_tensor("wdn", [128, NSD * EPC, HT, 128], bf16))
        x_sb = st.enter_context(
            nc.sbuf_tensor("x_sb", [128, EPC, HT, pio], bf16))
        g_sb = st.enter_context(
            nc.sbuf_tensor("g_sb", [128, EPC, HT, pio], f32))
        i_sb = st.enter_context(
            nc.sbuf_tensor("i_sb", [128, EPC, HT, pio], bf16))
        o_sb = st.enter_context(
            nc.sbuf_tensor("o_sb", [128, EPC, HT, pio], bf16))
        # one 8-bank PSUM pool; only one accumulation group may be open
        # per bank, so gate/up/down reuse banks with explicit WAR waits
        p8 = st.enter_context(nc.psum_tensor("p8", [128, 8, 512], f32))

        def reg(i):
            return p8[:, i, 0:pio]

        # kernel issues no GpSimd work: skip its expensive DGE drain
        # and use the cheaper sem-only end barrier
        with nc.Block(no_gpsimd_drain=True) as block:

            def w_cfg(eng, k):
                e, kind, lo, hi = chunks[k]
                if kind < 2:
                    eng.dma_start(wgu[:, e * 2 + kind, lo:hi],
                                  w[e, kind, :, lo:hi, :]
                                  ).then_inc(s_ws[k], 16)
                else:
                    eng.dma_start(
                        wdn[:, e * NSD + lo:e * NSD + hi],
                        wd[e, :, lo:hi]).then_inc(s_ws[k], 16)

            @block.sync
            def _(sync):
                # x (both experts, one 4.6 KB-run DMA) first - the PE's
                # first dependency - then the weight stream: every config
                # pre-issued, no pacing
                sync.dma_start(x_sb[:, :], xt[:, :]).then_inc(s_x[0], 16)
                for k in range(NW):
                    w_cfg(sync, k)
                # final j-tile ships from the (by now idle) SP queue so it
                # doesn't wait behind the Act queue's previous store
                sync.wait_ge(s_c, 8 * EPC)
                sync.dma_start(out[EPC - 1, :, HT - 1:HT, :],
                               o_sb[:, EPC - 1, HT - 1:HT]).then_inc(s_o2, 16)

            @block.tensor
            def _(tensor):
                tensor.wait_ge(s_x[0], 16)
                for e in range(EPC):
                    # gate, h-outer, chasing chunk arrival
                    for h in range(HT):
                        blk = hchunk[(e, 0, h)]
                        if h == blk[0]:
                            tensor.wait_ge(
                                s_ws[cidx[(e, 0) + blk]], 16)
                        for i in range(HT):
                            if h == 0 and e > 0:
                                # bank i WAR: expert e-1's down j-tile i
                                # must be copied out first
                                tensor.wait_ge(s_c, 8 * (e - 1) + i + 1)
                            mm = tensor.matmul(
                                reg(i), wgu[:, e * 2, h, ts(i, 128)],
                                x_sb[:, e, h],
                                start=(h == 0), stop=(h == HT - 1))
                            if h == HT - 1:
                                mm.then_inc(s_g)
                    # up (reuses gate's banks; region i waits silu[i])
                    for h in range(HT):
                        blk = hchunk[(e, 1, h)]
                        if h == blk[0]:
                            tensor.wait_ge(
                                s_ws[cidx[(e, 1) + blk]], 16)
                        for i in range(HT):
                            if h == 0:
                                tensor.wait_ge(s_s, 8 * e + i + 1)
                            mm = tensor.matmul(
                                reg(i), wgu[:, e * 2 + 1, h, ts(i, 128)],
                                x_sb[:, e, h],
                                start=(h == 0), stop=(h == HT - 1))
                            if h == HT - 1:
                                mm.then_inc(s_u)
                    # down, j-block chunks (bank j <- output j-tile). The
                    # first j-tile chases the DVE mul chain per-k (mul[k]
                    # wrote i_sb[k] and freed bank k) instead of waiting
                    # for all 8 muls; the second block carries the full
                    # wait, later blocks are implicitly ordered behind it.
                    for bi, (j0, j1) in enumerate(dblk(e)):
                        tensor.wait_ge(s_ws[cidx[(e, 2, j0, j1)]], 16)
                        if bi == 1:
                            tensor.wait_ge(s_m, 8 * (e + 1))
                        for j in range(j0, j1):
                            for k in range(HT):
                                if bi == 0 and j == j0:
                                    tensor.wait_ge(s_m, 8 * e + k + 1)
                                mm = tensor.matmul(
                                    p8[:, j, 0:pio],
                                    wdn[:, e * NSD + j, k, :],
                                    i_sb[:, e, k],
                                    start=(k == 0), stop=(k == HT - 1))
                                if k == HT - 1:
                                    mm.then_inc(s_d)

            @block.scalar
            def _(scalar):
                n_st = 0
                for e in range(EPC):
                    for i in range(HT):
                        scalar.wait_ge(s_g, 8 * e + i + 1)
                        scalar.activation(g_sb[:, e, i], reg(i),
                                          SILU).then_inc(s_s)
                    # stores: one 1728 B-run store for j0-5, then the
                    # 6..8 tail (the last expert's final j-tile ships
                    # from the SP queue instead)
                    jsplit = ([(0, 6), (6, 8)] if e < EPC - 1
                              else [(0, 6), (6, 7)])
                    for (j0, j1) in jsplit:
                        scalar.wait_ge(s_c, 8 * e + j1)
                        scalar.dma_start(
                            out[e, :, j0:j1, :],
                            o_sb[:, e, j0:j1]).then_inc(s_o, 16)
                        n_st += 1
                scalar.wait_ge(s_o, 16 * n_st)       # drain output stores
                scalar.wait_ge(s_o2, 16)

            @block.vector
            def _(vector):
                for e in range(EPC):
                    for i in range(HT):
                        vector.wait_ge(s_s, 8 * e + i + 1)
                        vector.wait_ge(s_u, 8 * e + i + 1)
                        vector.tensor_mul(i_sb[:, e, i], g_sb[:, e, i],
                                          reg(i)).then_inc(s_m)
                    for j in range(HT):
                        vector.wait_ge(s_d, 8 * e + j + 1)
                        vector.tensor_copy(o_sb[:, e, j],
                                           p8[:, j, 0:pio]).then_inc(s_c)

    nc.compile()
    return nc


def _get_nc(pio: int):
    if pio not in _NC_CACHE:
        _NC_CACHE[pio] = _build_nc_raw(pio)
    return _NC_CACHE[pio]


_ROUND_CAP = 160          # max tokens/expert per round (3 PSUM regions/bank)


def _kernel_once(x, expert_indices, gate_proj, up_proj, down_proj):
    import ml_dtypes
    from concourse.bass_utils import run_bass_kernel_spmd

    bf16 = np.dtype(ml_dtypes.bfloat16)
    x = np.ascontiguousarray(x, dtype=np.float32)
    b, s, h = x.shape
    assert (h, gate_proj.shape) == (H, (E, H, H)), (x.shape, gate_proj.shape)

    n = b * s
    xf = x.reshape(n, h)
    idx = np.asarray(expert_indices).reshape(n).astype(np.int64)

    order = np.argsort(idx, kind="stable")       # token ids grouped by expert
    counts = np.bincount(idx, minlength=E)
    starts = np.zeros(E + 1, dtype=np.int64)
    np.cumsum(counts, out=starts[1:])
    maxc = int(counts.max())
    assert maxc <= _ROUND_CAP
    pio = max(16, 16 * math.ceil(maxc / 16))

    # per-core weight packing (bf16, partition-major)
    wr = np.stack([gate_proj, up_proj], axis=1).astype(bf16) \
        .reshape(N_CORES, EPC, 2, HT, 128, H).transpose(0, 1, 2, 4, 3, 5)
    wdr = np.ascontiguousarray(down_proj.transpose(0, 2, 1)).astype(bf16) \
        .reshape(N_CORES, EPC, HT, 128, NSD, 128).transpose(0, 1, 3, 4, 2, 5)
    in_maps = []
    tok_ids = []
    for c in range(N_CORES):
        xt_c = np.zeros((EPC, H, pio), dtype=np.float32)
        toks = []
        for le in range(EPC):
            e = c * EPC + le
            te = order[starts[e]:starts[e + 1]]
            toks.append(te)
            xt_c[le, :, :len(te)] = xf[te].T
        tok_ids.append(toks)
        in_maps.append({
            "w": np.ascontiguousarray(wr[c]),
            "wd": np.ascontiguousarray(wdr[c]),
            # device xt is [128, EPC, HT, pio] (partition-major)
            "xt": xt_c.astype(bf16).reshape(EPC, HT, 128, pio)
                  .transpose(2, 0, 1, 3).copy(),
        })

    nc = _get_nc(pio)
    res = run_bass_kernel_spmd(nc, in_maps, core_ids=list(range(N_CORES)))

    out = np.empty((n, h), dtype=np.float32)
    for c in range(N_CORES):
        o = res.results[c]["out"]                # [EPC, 128, HT, pio] bf16
        for le in range(EPC):
            te = tok_ids[c][le]
            oe = np.asarray(o[le]).astype(np.float32) \
                .transpose(1, 0, 2).reshape(h, pio)      # [H, pio]
            out[te] = oe[:, :len(te)].T
    return out.reshape(b, s, h)


def kernel(x, expert_indices, gate_proj, up_proj, down_proj):
    """Full-input -> full-output entry point.

    Tokens-per-expert above _ROUND_CAP (pathological skew; PSUM bound)
    are handled by running the device kernel in multiple rounds over
    disjoint token slices - outputs are per-token independent."""
    idx = np.asarray(expert_indices)
    counts = np.bincount(idx.reshape(-1).astype(np.int64), minlength=E)
    if counts.max() <= _ROUND_CAP:
        return _kernel_once(x, expert_indices, gate_proj, up_proj, down_proj)

    b, s, h = x.shape
    n = b * s
    xf = np.ascontiguousarray(x, dtype=np.float32).reshape(n, h)
    idxf = idx.reshape(n).astype(np.int64)
    order = np.argsort(idxf, kind="stable")
    starts = np.zeros(E + 1, dtype=np.int64)
    np.cumsum(np.bincount(idxf, minlength=E), out=starts[1:])
    out = np.empty((n, h), dtype=np.float32)
    rounds = math.ceil(counts.max() / _ROUND_CAP)
    for r in range(rounds):
        sel = np.concatenate([
            order[starts[e] + r * _ROUND_CAP:
                  min(starts[e] + (r + 1) * _ROUND_CAP, starts[e + 1])]
            for e in range(E)])
        if not len(sel):
            continue
        xr = xf[sel].reshape(1, len(sel), h)
        ir = idxf[sel].reshape(1, len(sel))
        out[sel] = _kernel_once(
            xr, ir, gate_proj, up_proj, down_proj).reshape(len(sel), h)
    return out.reshape(b, s, h)


# revision 21
# speedup vs baseline: 1.1485x; 1.0207x over previous
"""Expert-parallel MoE MLP kernel for Trainium2 (8 NeuronCores).

Problem: x[B=2,S=1024,H=1024] f32, expert_indices[B,S] int, 16 experts,
gate/up_proj[E,H,I], down_proj[E,I,H] (H=I=1024):
    out[n] = silu(x_n @ Wg[e_n]) * (x_n @ Wu[e_n]) @ Wd[e_n].T

Sharding: expert parallelism - core c owns experts {2c, 2c+1}. The host
groups tokens by expert (the "all-to-all dispatch" runs on host since the
kernel contract is full-input -> full-output), pads each expert's token
block to a 16-multiple capacity, and each core runs dense per-expert GEMMs.

All operands are bf16 (rel err ~4e-3 vs the 2e-2 gate): 12.6 MB of
mandatory weight traffic per core.

The device program is RAW bass (no Tile framework). Profile-driven
design (v5):
  - the kernel is bound by SDMA per-engine line rate (~23 GB/s/engine at
    2 KB descriptors, 25.3 at 4 KB, 25.9 at 8 KB; descriptor = the
    per-partition contiguous DRAM run). The weight bulk streams as 1 MB
    chunks with 8 KB runs; fine grain only where the pipeline needs it:
    expert 0's first gate chunks are 0.5 MB so the PE starts early, and
    the last expert's down stream ends in single-j-tile chunks so the
    post-stream dependency is just 8 matmuls + 1 PSUM copy + 1 36 KB
    store
  - every weight chunk has its OWN completion semaphore and ALL configs
    are pre-issued on the SP ring with no pacing waits (each chunk has a
    dedicated SBUF slot -> no WAR hazard; per-ring FIFO keeps order) -
    the stream runs gapless at line rate
  - x (both experts, one 4.6 KB-run DMA) is SP-ring entry 0: the PE's
    first dependency, done ~4 us in
  - PE chases the stream h-outer; one 8-bank PSUM pool: gate accumulates
    into 8 banks, up reuses them after per-bank silu consumption, down
    gets bank j per output j-tile. The first down j-tile accumulates
    k-tiles in DVE-mul completion order (per-k waits) so the PE overlaps
    the serial 8-mul chain instead of idling ~2 us behind it
  - output stores ride the Act queue as one 1728 B-run store for j0-5
    plus a small tail; the final j-tile store goes on the (by then idle)
    SP queue
"""

import math

import numpy as np

E = 16
H = 1024
HT = 8           # H / 128 partition tiles
N_CORES = 8
EPC = E // N_CORES   # experts per core
NSD = 8          # down_proj j-tiles per expert (chunked into j-blocks)

# chunk h/j-blocks, chosen for >=8 KB per-partition DRAM runs (higher
# SDMA per-engine line rate) except where the pipeline needs fine grain:
# the first gate chunks of expert 0 are small so the PE starts early,
# and the LAST expert's down stream ends with single-tile blocks so the
# kernel tail after the last weight byte is minimal
_GBLK0 = [(0, 2), (2, 4), (4, 6), (6, 8)]        # gate, expert 0
_GBLK = [(0, 2), (2, 4), (4, 6), (6, 8)]         # gate, experts 1+ / up
_DBLK = [(0, 2), (2, 4), (4, 6), (6, 8)]         # down, experts 0..EPC-2
_DBLK_LAST = [(0, 2), (2, 4), (4, 6), (6, 7), (7, 8)]   # down, expert EPC-1

_NC_CACHE = {}


def _build_nc_raw(pio: int, act: str = "Silu"):
    """Raw-bass SPMD program. pio: padded token count, multiple of 16,
    <= 160."""
    from concourse import bacc, mybir
    from concourse.bass import ts

    f32 = mybir.dt.float32
    bf16 = mybir.dt.bfloat16
    SILU = getattr(mybir.ActivationFunctionType, act)
    assert 3 * pio * 4 <= 2048

    nc = bacc.Bacc("TRN2", target_bir_lowering=False, debug=False,
                   num_devices=N_CORES)
    w = nc.dram_tensor("w", [EPC, 2, 128, HT, H], bf16, kind="ExternalInput")
    wd = nc.dram_tensor("wd", [EPC, 128, NSD, HT, 128], bf16,
                        kind="ExternalInput")
    xt = nc.dram_tensor("xt", [128, EPC, HT, pio], bf16, kind="ExternalInput")
    out = nc.dram_tensor("out", [EPC, 128, HT, pio], bf16,
                         kind="ExternalOutput")

    def gblk(e):
        return _GBLK0 if e == 0 else _GBLK

    def dblk(e):
        return _DBLK_LAST if e == EPC - 1 else _DBLK

    # flat chunk list in stream order; values: (expert, kind, lo, hi)
    chunks = []
    for e in range(EPC):
        for (lo, hi) in gblk(e):
            chunks.append((e, 0, lo, hi))                  # gate h-tiles
        for (lo, hi) in _GBLK:
            chunks.append((e, 1, lo, hi))                  # up h-tiles
        for (j0, j1) in dblk(e):
            chunks.append((e, 2, j0, j1))                  # down j-tiles
    NW = len(chunks)
    cidx = {c: k for k, c in enumerate(chunks)}
    # h -> owning chunk (lo, hi), per expert and projection
    hchunk = {}
    for e in range(EPC):
        for (lo, hi) in gblk(e):
            for h in range(lo, hi):
                hchunk[(e, 0, h)] = (lo, hi)
        for (lo, hi) in _GBLK:
            for h in range(lo, hi):
                hchunk[(e, 1, h)] = (lo, hi)

    import contextlib
    with contextlib.ExitStack() as st:
        s_ws = [st.enter_context(nc.semaphore(f"s_w{i}")) for i in range(NW)]
        s_x = [st.enter_context(nc.semaphore("s_x"))]
        s_g = st.enter_context(nc.semaphore("s_g"))   # gate region done (PE)
        s_u = st.enter_context(nc.semaphore("s_u"))   # up region done (PE)
        s_s = st.enter_context(nc.semaphore("s_s"))   # silu done (Act)
        s_m = st.enter_context(nc.semaphore("s_m"))   # inter mul done (DVE)
        s_d = st.enter_context(nc.semaphore("s_d"))   # down region done (PE)
        s_c = st.enter_context(nc.semaphore("s_c"))   # out copy done (DVE)
        s_o = st.enter_context(nc.semaphore("s_o"))   # out stores (+16 each)
        wgu = st.enter_context(
            nc.sbuf_tensor("wgu", [128, 2 * EPC, HT, H], bf16))
        wdn = st.enter_context(
            nc.sbuf_tensor("wdn", [128, NSD * EPC, HT, 128], bf16))
        x_sb = st.enter_context(
            nc.sbuf_tensor("x_sb", [128, EPC, HT, pio], bf16))
        g_sb = st.enter_context(
            nc.sbuf_tensor("g_sb", [128, EPC, HT, pio], f32))
        i_sb = st.enter_context(
            nc.sbuf_tensor("i_sb", [128, EPC, HT, pio], bf16))
        o_sb = st.enter_context(
            nc.sbuf_tensor("o_sb", [128, EPC, HT, pio], bf16))
        # one 8-bank PSUM pool; only one accumulation group may be open
        # per bank, so gate/up/down reuse banks with explicit WAR waits
        p8 = st.enter_context(nc.psum_tensor("p8", [128, 8, 512], f32))

        def reg(i):
            return p8[:, i, 0:pio]

        # kernel issues no GpSimd work: skip its expensive DGE drain
        # and use the cheaper sem-only end barrier
        with nc.Block(no_gpsimd_drain=True) as block:

            def w_cfg(eng, k):
                e, kind, lo, hi = chunks[k]
                if kind < 2:
                    eng.dma_start(wgu[:, e * 2 + kind, lo:hi],
                                  w[e, kind, :, lo:hi, :]
                                  ).then_inc(s_ws[k], 16)
                else:
                    eng.dma_start(
                        wdn[:, e * NSD + lo:e * NSD + hi],
                        wd[e, :, lo:hi]).then_inc(s_ws[k], 16)

            @block.sync
            def _(sync):
                # x (both experts, one 4.6 KB-run DMA) first - the PE's
                # first dependency - then the weight stream: every config
                # pre-issued, no pacing
                sync.dma_start(x_sb[:, :], xt[:, :]).then_inc(s_x[0], 16)
                for k in range(NW):
                    w_cfg(sync, k)
                # output stores ride the SAME ring BEHIND all weight
                # descriptors: ring FIFO gives the weight stream 100% of
                # the SDMA slots until it drains, and the stores (whose
                # copies are long ready) then flush in the shadow of the
                # PE tail. The last expert's j6/j7 tiles ship separately
                # so the post-stream dependency stays minimal.
                for e in range(EPC):
                    jsplit = ([(0, 8)] if e < EPC - 1
                              else [(0, 6), (6, 7), (7, 8)])
                    for (j0, j1) in jsplit:
                        sync.wait_ge(s_c, 8 * e + j1)
                        sync.dma_start(
                            out[e, :, j0:j1, :],
                            o_sb[:, e, j0:j1]).then_inc(s_o, 16)
                sync.wait_ge(s_o, 16 * (EPC - 1 + 3))   # drain stores

            @block.tensor
            def _(tensor):
                tensor.wait_ge(s_x[0], 16)
                for e in range(EPC):
                    # gate, h-outer, chasing chunk arrival
                    for h in range(HT):
                        blk = hchunk[(e, 0, h)]
                        if h == blk[0]:
                            tensor.wait_ge(
                                s_ws[cidx[(e, 0) + blk]], 16)
                        for i in range(HT):
                            if h == 0 and e > 0:
                                # bank i WAR: expert e-1's down j-tile i
                                # must be copied out first
                                tensor.wait_ge(s_c, 8 * (e - 1) + i + 1)
                            mm = tensor.matmul(
                                reg(i), wgu[:, e * 2, h, ts(i, 128)],
                                x_sb[:, e, h],
                                start=(h == 0), stop=(h == HT - 1))
                            if h == HT - 1:
                                mm.then_inc(s_g)
                    # up (reuses gate's banks; region i waits silu[i])
                    for h in range(HT):
                        blk = hchunk[(e, 1, h)]
                        if h == blk[0]:
                            tensor.wait_ge(
                                s_ws[cidx[(e, 1) + blk]], 16)
                        for i in range(HT):
                            if h == 0:
                                tensor.wait_ge(s_s, 8 * e + i + 1)
                            mm = tensor.matmul(
                                reg(i), wgu[:, e * 2 + 1, h, ts(i, 128)],
                                x_sb[:, e, h],
                                start=(h == 0), stop=(h == HT - 1))
                            if h == HT - 1:
                                mm.then_inc(s_u)
                    # down, j-block chunks (bank j <- output j-tile). The
                    # first j-tile chases the DVE mul chain per-k (mul[k]
                    # wrote i_sb[k] and freed bank k) instead of waiting
                    # for all 8 muls; the second block carries the full
                    # wait, later blocks are implicitly ordered behind it.
                    for bi, (j0, j1) in enumerate(dblk(e)):
                        tensor.wait_ge(s_ws[cidx[(e, 2, j0, j1)]], 16)
                        if bi == 1:
                            tensor.wait_ge(s_m, 8 * (e + 1))
                        for j in range(j0, j1):
                            for k in range(HT):
                                if bi == 0 and j == j0:
                                    tensor.wait_ge(s_m, 8 * e + k + 1)
                                mm = tensor.matmul(
                                    p8[:, j, 0:pio],
                                    wdn[:, e * NSD + j, k, :],
                                    i_sb[:, e, k],
                                    start=(k == 0), stop=(k == HT - 1))
                                if k == HT - 1:
                                    mm.then_inc(s_d)

            @block.scalar
            def _(scalar):
                for e in range(EPC):
                    for i in range(HT):
                        scalar.wait_ge(s_g, 8 * e + i + 1)
                        scalar.activation(g_sb[:, e, i], reg(i),
                                          SILU).then_inc(s_s)

            @block.vector
            def _(vector):
                for e in range(EPC):
                    for i in range(HT):
                        vector.wait_ge(s_s, 8 * e + i + 1)
                        vector.wait_ge(s_u, 8 * e + i + 1)
                        vector.tensor_mul(i_sb[:, e, i], g_sb[:, e, i],
                                          reg(i)).then_inc(s_m)
                    for j in range(HT):
                        vector.wait_ge(s_d, 8 * e + j + 1)
                        vector.tensor_copy(o_sb[:, e, j],
                                           p8[:, j, 0:pio]).then_inc(s_c)

    nc.compile()
    return nc


def _get_nc(pio: int):
    if pio not in _NC_CACHE:
        _NC_CACHE[pio] = _build_nc_raw(pio)
    return _NC_CACHE[pio]


_ROUND_CAP = 160          # max tokens/expert per round (3 PSUM regions/bank)


def _kernel_once(x, expert_indices, gate_proj, up_proj, down_proj):
    import ml_dtypes
    from concourse.bass_utils import run_bass_kernel_spmd

    bf16 = np.dtype(ml_dtypes.bfloat16)
    x = np.ascontiguousarray(x, dtype=np.float32)
    b, s, h = x.shape
    assert (h, gate_proj.shape) == (H, (E, H, H)), (x.shape, gate_proj.shape)

    n = b * s
    xf = x.reshape(n, h)
    idx = np.asarray(expert_indices).reshape(n).astype(np.int64)

    order = np.argsort(idx, kind="stable")       # token ids grouped by expert
    counts = np.bincount(idx, minlength=E)
    starts = np.zeros(E + 1, dtype=np.int64)
    np.cumsum(counts, out=starts[1:])
    maxc = int(counts.max())
    assert maxc <= _ROUND_CAP
    pio = max(16, 16 * math.ceil(maxc / 16))

    # per-core weight packing (bf16, partition-major)
    wr = np.stack([gate_proj, up_proj], axis=1).astype(bf16) \
        .reshape(N_CORES, EPC, 2, HT, 128, H).transpose(0, 1, 2, 4, 3, 5)
    wdr = np.ascontiguousarray(down_proj.transpose(0, 2, 1)).astype(bf16) \
        .reshape(N_CORES, EPC, HT, 128, NSD, 128).transpose(0, 1, 3, 4, 2, 5)
    in_maps = []
    tok_ids = []
    for c in range(N_CORES):
        xt_c = np.zeros((EPC, H, pio), dtype=np.float32)
        toks = []
        for le in range(EPC):
            e = c * EPC + le
            te = order[starts[e]:starts[e + 1]]
            toks.append(te)
            xt_c[le, :, :len(te)] = xf[te].T
        tok_ids.append(toks)
        in_maps.append({
            "w": np.ascontiguousarray(wr[c]),
            "wd": np.ascontiguousarray(wdr[c]),
            # device xt is [128, EPC, HT, pio] (partition-major)
            "xt": xt_c.astype(bf16).reshape(EPC, HT, 128, pio)
                  .transpose(2, 0, 1, 3).copy(),
        })

    nc = _get_nc(pio)
    res = run_bass_kernel_spmd(nc, in_maps, core_ids=list(range(N_CORES)))

    out = np.empty((n, h), dtype=np.float32)
    for c in range(N_CORES):
        o = res.results[c]["out"]                # [EPC, 128, HT, pio] bf16
        for le in range(EPC):
            te = tok_ids[c][le]
            oe = np.asarray(o[le]).astype(np.float32) \
                .transpose(1, 0, 2).reshape(h, pio)      # [H, pio]
            out[te] = oe[:, :len(te)].T
    return out.reshape(b, s, h)


def kernel(x, expert_indices, gate_proj, up_proj, down_proj):
    """Full-input -> full-output entry point.

    Tokens-per-expert above _ROUND_CAP (pathological skew; PSUM bound)
    are handled by running the device kernel in multiple rounds over
    disjoint token slices - outputs are per-token independent."""
    idx = np.asarray(expert_indices)
    counts = np.bincount(idx.reshape(-1).astype(np.int64), minlength=E)
    if counts.max() <= _ROUND_CAP:
        return _kernel_once(x, expert_indices, gate_proj, up_proj, down_proj)

    b, s, h = x.shape
    n = b * s
    xf = np.ascontiguousarray(x, dtype=np.float32).reshape(n, h)
    idxf = idx.reshape(n).astype(np.int64)
    order = np.argsort(idxf, kind="stable")
    starts = np.zeros(E + 1, dtype=np.int64)
    np.cumsum(np.bincount(idxf, minlength=E), out=starts[1:])
    out = np.empty((n, h), dtype=np.float32)
    rounds = math.ceil(counts.max() / _ROUND_CAP)
    for r in range(rounds):
        sel = np.concatenate([
            order[starts[e] + r * _ROUND_CAP:
                  min(starts[e] + (r + 1) * _ROUND_CAP, starts[e + 1])]
            for e in range(E)])
        if not len(sel):
            continue
        xr = xf[sel].reshape(1, len(sel), h)
        ir = idxf[sel].reshape(1, len(sel))
        out[sel] = _kernel_once(
            xr, ir, gate_proj, up_proj, down_proj).reshape(len(sel), h)
    return out.reshape(b, s, h)
